# revision 5
# baseline (speedup 1.0000x reference)
"""Causal single-head attention on 8 TRN2 NeuronCores.

Problem: x[4, 2048, 1024], Wq/Wk/Wv[1024, 1024] fp32.
  q,k,v = x@W*; scores = q@k^T; masked = scores*tril + (1-tril)*(-1e9)
  attn = softmax(masked/sqrt(1024)); out = attn@v.

Sharding: 2 cores per batch. Query rows are split into eight 256-row
blocks; parity-0 cores take blocks {0,2,4,6}, parity-1 {1,3,5,7}, so
each core's 4 slots attend to exactly (1,2,3,4) 512-wide key panels —
identical program on all 8 cores (SPMD), balanced causal work, no
collectives. Each core computes k/v projections for its whole batch
(k^T and v bounce through DRAM scratch), q projection for its 1024
rows, then block-wise masked softmax(QK^T/32)V. Matmuls run in
float32r (~13-bit mantissa, 4x faster than fp32 on the PE).

Host side: slices x per core, pre-transposes x and xq (so the kernel
needs no PE transposes for projections), builds additive causal mask
biases for each slot's last key panel, and scatters the per-core
outputs back into the full [4, 2048, 1024] tensor.
"""
import sys

if "/opt/trn_rl_repo" not in sys.path:
    sys.path.insert(0, "/opt/trn_rl_repo")

import numpy as np

import concourse.bass as bass
import concourse.tile as tile
from concourse import bacc, mybir
from concourse.bass_utils import run_bass_kernel_spmd
from concourse.masks import make_identity

dt = mybir.dt

B, S, D = 4, 2048, 1024
P = 128
NEG = -1.0e9
QBLK = 256            # query rows per slot
KPAN = 512            # key panel width
NSLOT = 4             # slots per core
COUNTS = (1, 2, 3, 4)  # key panels per slot (both parities)
SCALE = 1.0 / 32.0    # 1/sqrt(D)

_nc_cache = {}


def build_nc(reps=1):
    """Build the per-core Bass program (same NEFF for all 8 cores)."""
    nc = bacc.Bacc(None, target_bir_lowering=False, debug=False)

    xt = nc.dram_tensor("xt", [D, S], dt.float32, kind="ExternalInput")
    xqt = nc.dram_tensor("xqt", [D, NSLOT * QBLK], dt.float32, kind="ExternalInput")
    wq = nc.dram_tensor("wq", [D, D], dt.float32, kind="ExternalInput")
    wk = nc.dram_tensor("wk", [D, D], dt.float32, kind="ExternalInput")
    wv = nc.dram_tensor("wv", [D, D], dt.float32, kind="ExternalInput")
    # additive causal bias for each slot's LAST key panel, laid out
    # [p, slot, qsub, key] with q-local row = qsub*128 + p
    mb = nc.dram_tensor("mb", [P, NSLOT, 2, KPAN], dt.float32, kind="ExternalInput")
    out = nc.dram_tensor("out", [NSLOT * QBLK, D], dt.float32, kind="ExternalOutput")

    # DRAM scratch (f32r-rounded bits) for q^T and k^T; v stays in SBUF
    qt_d = nc.dram_tensor("qt_d", [P, 8, NSLOT * QBLK], dt.float32r)
    kt_d = nc.dram_tensor("kt_d", [P, 8, S], dt.float32r)

    DC = D // P  # 8 contraction chunks

    def proj_matmuls(psum_t, lhs_r, rhs_r):
        for dc in range(DC):
            nc.tensor.matmul(
                psum_t, lhs_r[:, dc], rhs_r[:, dc],
                start=(dc == 0), stop=(dc == DC - 1),
            )

    def load_round(pool, dram_rearr, shape, tag, stage_pool, stage_shape,
                   nhalf, name):
        """DMA fp32 from DRAM in halves and round into one f32r tile."""
        t = pool.tile(shape, dt.float32r, tag=tag, name=name)
        hw = shape[-1] // nhalf
        for h in range(nhalf):
            st = stage_pool.tile(stage_shape, dt.float32, tag="st",
                                 name=f"{name}_st{h}")
            nc.sync.dma_start(st[:], dram_rearr[:, :, h * hw:(h + 1) * hw])
            nc.vector.tensor_copy(t[:, :, h * hw:(h + 1) * hw], st[:])
        return t

    with tile.TileContext(nc) as tc:
        with (
            tc.tile_pool(name="const", bufs=1) as const,
            tc.tile_pool(name="persist", bufs=1) as persist,
        ):
            ident = const.tile([P, P], dt.float32)
            make_identity(nc, ident)
            # v[key, dout] stays resident across phases V and A
            v_res = persist.tile([P, S // P, D], dt.float32r)

            def body():
                # ---- Phase Q: q^T projection -> qt_d ----
                with (
                    tc.tile_pool(name="qstage", bufs=2) as qstage,
                    tc.tile_pool(name="qround", bufs=1) as qround,
                    tc.tile_pool(name="qost", bufs=4) as qost,
                    tc.tile_pool(name="psum_q", bufs=4, space="PSUM") as psum_q,
                ):
                    xq_r = load_round(
                        qround, xqt.rearrange("(dc p) t -> p dc t", p=P),
                        [P, DC, 1024], "xq", qstage, [P, DC, 512], 2, "xq_r")
                    wq_r = load_round(
                        qround, wq.rearrange("(dc p) m -> p dc m", p=P),
                        [P, DC, 1024], "wq", qstage, [P, DC, 512], 2, "wq_r")
                    for do in range(DC):
                        for th in range(2):
                            ps = psum_q.tile([P, 512], dt.float32, tag="pp")
                            proj_matmuls(
                                ps,
                                wq_r[:, :, do * P:(do + 1) * P],
                                xq_r[:, :, th * 512:(th + 1) * 512],
                            )
                            st = qost.tile([P, 512], dt.float32r, tag="qo")
                            nc.vector.tensor_copy(st[:], ps[:])
                            nc.sync.dma_start(
                                qt_d[:, do, th * 512:(th + 1) * 512], st[:])

                # ---- Phases K, V ----
                with tc.tile_pool(name="xtpool", bufs=1) as xtpool:
                    with tc.tile_pool(name="xstage", bufs=1) as xstage:
                        xt_r = load_round(
                            xtpool, xt.rearrange("(dc p) t -> p dc t", p=P),
                            [P, DC, S], "xtr", xstage, [P, DC, 1024], 2, "xt_r")

                    # K: kt[dout, key] -> kt_d
                    with (
                        tc.tile_pool(name="wkpool", bufs=1) as wkpool,
                        tc.tile_pool(name="wkstage", bufs=1) as wkstage,
                        tc.tile_pool(name="kost", bufs=4) as kost,
                        tc.tile_pool(name="psum_k", bufs=4, space="PSUM") as psum_k,
                    ):
                        wk_r = load_round(
                            wkpool, wk.rearrange("(dc p) m -> p dc m", p=P),
                            [P, DC, 1024], "wk", wkstage, [P, DC, 512], 2, "wk_r")
                        for do in range(DC):
                            for kq in range(S // 512):
                                ps = psum_k.tile([P, 512], dt.float32, tag="pp")
                                proj_matmuls(
                                    ps,
                                    wk_r[:, :, do * P:(do + 1) * P],
                                    xt_r[:, :, kq * 512:(kq + 1) * 512],
                                )
                                st = kost.tile([P, 512], dt.float32r, tag="ko")
                                nc.vector.tensor_copy(st[:], ps[:])
                                nc.sync.dma_start(
                                    kt_d[:, do, kq * 512:(kq + 1) * 512], st[:])

                    # V: v[key, dout] -> v_res (SBUF resident)
                    with (
                        tc.tile_pool(name="wvpool", bufs=1) as wvpool,
                        tc.tile_pool(name="wvstage", bufs=1) as wvstage,
                        tc.tile_pool(name="psum_v", bufs=4, space="PSUM") as psum_v,
                    ):
                        wv_r = load_round(
                            wvpool, wv.rearrange("(dc p) m -> p dc m", p=P),
                            [P, DC, 1024], "wv", wvstage, [P, DC, 512], 2, "wv_r")
                        for kc in range(S // P):
                            for dh in range(2):
                                ps = psum_v.tile([P, 512], dt.float32, tag="pp")
                                proj_matmuls(
                                    ps,
                                    xt_r[:, :, kc * P:(kc + 1) * P],
                                    wv_r[:, :, dh * 512:(dh + 1) * 512],
                                )
                                nc.vector.tensor_copy(
                                    v_res[:, kc, dh * 512:(dh + 1) * 512],
                                    ps[:])

                # ---- Phase A: blockwise masked softmax(QK^T/32) V ----
                with (
                    tc.tile_pool(name="qtpool", bufs=1) as qtpool,
                    tc.tile_pool(name="attn", bufs=1) as attn,
                    tc.tile_pool(name="ktpool", bufs=1) as ktpool,
                    tc.tile_pool(name="ptpool", bufs=1) as ptpool,
                    tc.tile_pool(name="mpool", bufs=2) as mpool,
                    tc.tile_pool(name="opool", bufs=2) as opool,
                    tc.tile_pool(name="small", bufs=24) as small,
                    tc.tile_pool(name="psum_s", bufs=2, space="PSUM") as psum_s,
                    tc.tile_pool(name="psum_t", bufs=2, space="PSUM") as psum_t,
                    tc.tile_pool(name="psum_c", bufs=4, space="PSUM") as psum_c,
                ):
                    qt_r = qtpool.tile([P, DC, NSLOT * QBLK], dt.float32r)
                    nc.sync.dma_start(qt_r[:], qt_d[:])
                    scores = [
                        attn.tile([P, 2, (s + 1) * KPAN], dt.float32,
                                  tag=f"sc{s}", name=f"scores{s}")
                        for s in range(NSLOT)
                    ]
                    masksl = [
                        mpool.tile([P, 2, KPAN], dt.float32, tag="mk",
                                   name=f"mask{s}")
                        for s in range(NSLOT)
                    ]
                    # panel-major scores: k^T panel read once
                    for p in range(NSLOT):
                        nc.sync.dma_start(masksl[p][:], mb[:, p])
                        ktp = ktpool.tile([P, DC, KPAN], dt.float32r, tag="kt")
                        nc.sync.dma_start(
                            ktp[:], kt_d[:, :, p * KPAN:(p + 1) * KPAN])
                        for s in range(p, NSLOT):
                            for qs in range(2):
                                ps = psum_s.tile([P, KPAN], dt.float32, tag="ps")
                                for dc in range(DC):
                                    nc.tensor.matmul(
                                        ps,
                                        qt_r[:, dc,
                                             s * QBLK + qs * P:
                                             s * QBLK + (qs + 1) * P],
                                        ktp[:, dc],
                                        start=(dc == 0), stop=(dc == DC - 1),
                                    )
                                dst = scores[s][:, qs, p * KPAN:(p + 1) * KPAN]
                                if p == s:  # this slot's last panel: add mask
                                    nc.vector.tensor_tensor(
                                        dst, ps[:], masksl[s][:, qs, :],
                                        op=mybir.AluOpType.add)
                                else:
                                    nc.vector.tensor_copy(dst, ps[:])

                    for s in range(NSLOT):
                        W = (s + 1) * KPAN
                        KC = W // P
                        rinvs = []
                        for qs in range(2):
                            row = scores[s][:, qs, :]
                            mx = small.tile([P, 1], dt.float32, tag="mx")
                            nc.vector.reduce_max(
                                mx, row, axis=mybir.AxisListType.X)
                            bias_act = small.tile([P, 1], dt.float32, tag="ba")
                            nc.vector.tensor_scalar_mul(bias_act, mx, -SCALE)
                            lsum = small.tile([P, 1], dt.float32, tag="ls")
                            nc.scalar.activation(
                                out=row, in_=row,
                                func=mybir.ActivationFunctionType.Exp,
                                bias=bias_act, scale=SCALE, accum_out=lsum)
                            rinv = small.tile([P, 1], dt.float32, tag="ri")
                            nc.vector.reciprocal(rinv, lsum)
                            rinvs.append(rinv)
                        # transpose p -> pT (f32r) for the AV matmul
                        pt = ptpool.tile([P, 16, QBLK], dt.float32r, tag="pt")
                        for kc in range(KC):
                            for qs in range(2):
                                tps = psum_t.tile([P, P], dt.float32, tag="tp")
                                nc.tensor.transpose(
                                    tps,
                                    scores[s][:, qs, kc * P:(kc + 1) * P],
                                    ident)
                                nc.vector.tensor_copy(
                                    pt[:, kc, qs * P:(qs + 1) * P], tps[:])
                        # AV: ctx[q, dout] accumulated over key chunks
                        ctx = [[psum_c.tile([P, 512], dt.float32, tag="ctx",
                                             name=f"ctx{s}_{qs}_{dh}")
                                for dh in range(2)] for qs in range(2)]
                        for kc in range(KC):
                            for qs in range(2):
                                for dh in range(2):
                                    nc.tensor.matmul(
                                        ctx[qs][dh],
                                        pt[:, kc, qs * P:(qs + 1) * P],
                                        v_res[:, kc, dh * 512:(dh + 1) * 512],
                                        start=(kc == 0), stop=(kc == KC - 1),
                                    )
                        for qs in range(2):
                            for dh in range(2):
                                oc = opool.tile([P, 512], dt.float32, tag="oc")
                                nc.vector.tensor_tensor(
                                    oc[:], ctx[qs][dh],
                                    rinvs[qs][:].to_broadcast((P, 512)),
                                    op=mybir.AluOpType.mult)
                                nc.sync.dma_start(
                                    out[s * QBLK + qs * P:
                                        s * QBLK + (qs + 1) * P,
                                        dh * 512:(dh + 1) * 512],
                                    oc[:])

            if reps > 1:
                with tc.For_i(0, reps):
                    body()
            else:
                body()

    nc.finalize()
    return nc


def make_core_inputs(x, Wq, Wk, Wv):
    """Slice/transform full inputs into 8 per-core input dicts."""
    in_maps = []
    qi = np.arange(QBLK)
    for c in range(8):
        b, par = c // 2, c % 2
        blocks = [2 * j + par for j in range(NSLOT)]
        xb = x[b]  # [S, D]
        xt = np.ascontiguousarray(xb.T)  # [D, S]
        qrows = np.concatenate(
            [np.arange(QBLK * blk, QBLK * (blk + 1)) for blk in blocks])
        xqt = np.ascontiguousarray(xb[qrows].T)  # [D, 1024]
        # additive bias for each slot's last key panel
        mb = np.zeros((NSLOT, 2, P, KPAN), np.float32)
        for s in range(NSLOT):
            bs = blocks[s]
            kidx = (COUNTS[s] - 1) * KPAN + np.arange(KPAN)[None, :]
            qidx = (QBLK * bs + qi)[:, None]
            bias = np.where(kidx <= qidx, 0.0, NEG).astype(np.float32)
            mb[s] = bias.reshape(2, P, KPAN)
        mb = np.ascontiguousarray(mb.transpose(2, 0, 1, 3))  # [P, slot, qs, k]
        in_maps.append({
            "xt": xt, "xqt": xqt, "wq": Wq, "wk": Wk, "wv": Wv, "mb": mb,
        })
    return in_maps


def assemble_output(results):
    out = np.empty((B, S, D), np.float32)
    for c in range(8):
        b, par = c // 2, c % 2
        blocks = [2 * j + par for j in range(NSLOT)]
        o = results[c]["out"]  # [1024, D]
        for s, blk in enumerate(blocks):
            out[b, QBLK * blk:QBLK * (blk + 1)] = o[QBLK * s:QBLK * (s + 1)]
    return out


def kernel(x, Wq, Wk, Wv):
    x = np.asarray(x, np.float32)
    Wq = np.asarray(Wq, np.float32)
    Wk = np.asarray(Wk, np.float32)
    Wv = np.asarray(Wv, np.float32)
    if "nc" not in _nc_cache:
        _nc_cache["nc"] = build_nc()
    nc = _nc_cache["nc"]
    in_maps = make_core_inputs(x, Wq, Wk, Wv)
    res = run_bass_kernel_spmd(nc, in_maps, core_ids=list(range(8)))
    return assemble_output(res.results)


# revision 11
# speedup vs baseline: 1.0452x; 1.0452x over previous
"""Causal single-head attention on 8 TRN2 NeuronCores.

Problem: x[4, 2048, 1024], Wq/Wk/Wv[1024, 1024] fp32.
  q,k,v = x@W*; scores = q@k^T; masked = scores*tril + (1-tril)*(-1e9)
  attn = softmax(masked/sqrt(1024)); out = attn@v.

Sharding: 2 cores per batch. Query rows are split into eight 256-row
blocks; parity-0 cores take blocks {0,2,4,6}, parity-1 {1,3,5,7}, so
each core's 4 slots attend to exactly (1,2,3,4) 512-wide key panels —
identical program on all 8 cores (SPMD), balanced causal work, no
collectives. Each core computes k/v projections for its whole batch
(k^T and v bounce through DRAM scratch), q projection for its 1024
rows, then block-wise masked softmax(QK^T/32)V. Matmuls run in
float32r (~13-bit mantissa, 4x faster than fp32 on the PE).

Host side: slices x per core, pre-transposes x and xq (so the kernel
needs no PE transposes for projections), builds additive causal mask
biases for each slot's last key panel, and scatters the per-core
outputs back into the full [4, 2048, 1024] tensor.
"""
import sys

if "/opt/trn_rl_repo" not in sys.path:
    sys.path.insert(0, "/opt/trn_rl_repo")

import numpy as np

import concourse.bass as bass
import concourse.tile as tile
from concourse import bacc, mybir
from concourse.bass_utils import run_bass_kernel_spmd
from concourse.masks import make_identity

dt = mybir.dt

B, S, D = 4, 2048, 1024
P = 128
NEG = -1.0e9
QBLK = 256            # query rows per slot
KPAN = 512            # key panel width
NSLOT = 4             # slots per core
COUNTS = (1, 2, 3, 4)  # key panels per slot (both parities)
SCALE = 1.0 / 32.0    # 1/sqrt(D)

_nc_cache = {}


def round_f32r(a):
    """Host replica of the DVE fp32->float32r rounding: round-to-nearest-even
    to 11 mantissa bits (drop 12). Verified bit-exact vs hardware."""
    u = np.ascontiguousarray(a, np.float32).view(np.uint32).astype(np.uint64)
    half = np.uint64(1 << 11)
    tie = ((u >> np.uint64(12)) & np.uint64(1)) ^ np.uint64(1)
    r = (u + half - tie) & np.uint64(0xFFFFF000)
    return r.astype(np.uint32).view(np.float32)


def build_nc(reps=1):
    """Build the per-core Bass program (same NEFF for all 8 cores).

    All matmuls run in float32r. The host pre-rounds every input to f32r
    bits, so inputs DMA straight into f32r tiles with no on-device
    rounding pass. Phases: Q (q^T, kept resident) -> fused K+V streaming
    over x^T chunks (k^T panels bounce through DRAM, v resident) ->
    panel-major masked softmax(QK^T/32) V.
    """
    nc = bacc.Bacc(None, target_bir_lowering=False, debug=False)

    # all big inputs arrive pre-rounded to f32r bit patterns
    xt = nc.dram_tensor("xt", [D, S], dt.float32r, kind="ExternalInput")
    xqt = nc.dram_tensor("xqt", [D, NSLOT * QBLK], dt.float32r,
                         kind="ExternalInput")
    wq = nc.dram_tensor("wq", [D, D], dt.float32r, kind="ExternalInput")
    wk = nc.dram_tensor("wk", [D, D], dt.float32r, kind="ExternalInput")
    wv = nc.dram_tensor("wv", [D, D], dt.float32r, kind="ExternalInput")
    # additive causal bias for each slot's LAST key panel, laid out
    # [p, slot, qsub, key] with q-local row = qsub*128 + p
    mb = nc.dram_tensor("mb", [P, NSLOT, 2, KPAN], dt.float32,
                        kind="ExternalInput")
    out = nc.dram_tensor("out", [NSLOT * QBLK, D], dt.float32,
                         kind="ExternalOutput")

    # k^T DRAM bounce, one tensor per 512-key panel (fine-grained deps)
    kt_ds = [nc.dram_tensor(f"kt_d{p}", [P, 8, KPAN], dt.float32r)
             for p in range(NSLOT)]

    DC = D // P  # 8 contraction chunks
    CH = 256     # x^T streaming chunk width (keys)

    def proj_matmuls(psum_t, lhs_r, rhs_r):
        for dc in range(DC):
            nc.tensor.matmul(
                psum_t, lhs_r[:, dc], rhs_r[:, dc],
                start=(dc == 0), stop=(dc == DC - 1),
            )

    with tile.TileContext(nc) as tc:
        with (
            tc.tile_pool(name="vres", bufs=1) as vres,
            tc.tile_pool(name="qtres", bufs=1) as qtres,
        ):
            # v[key, dout] and q^T, resident through the attention phase
            v_res = vres.tile([P, S // P, D], dt.float32r)
            qt_r = qtres.tile([P, DC, NSLOT * QBLK], dt.float32r)

            def body():
                # ---- Phase Q: q^T -> qt_r (SBUF resident) ----
                with (
                    tc.tile_pool(name="wqpool", bufs=1) as wqpool,
                    tc.tile_pool(name="xqpool", bufs=1) as xqpool,
                    tc.tile_pool(name="psum_q", bufs=4, space="PSUM") as psum_q,
                ):
                    wq_r = wqpool.tile([P, DC, D], dt.float32r)
                    xq_r = xqpool.tile([P, DC, NSLOT * QBLK], dt.float32r)
                    wqa = wq.rearrange("(dc p) m -> p dc m", p=P)
                    xqa = xqt.rearrange("(dc p) t -> p dc t", p=P)
                    nc.sync.dma_start(xq_r[:, :, 0:512], xqa[:, :, 0:512])
                    for do in range(DC):
                        sl = slice(do * P, (do + 1) * P)
                        nc.sync.dma_start(wq_r[:, :, sl], wqa[:, :, sl])
                    nc.sync.dma_start(xq_r[:, :, 512:1024], xqa[:, :, 512:1024])
                    for th in range(2):
                        for do in range(DC):
                            ps = psum_q.tile([P, 512], dt.float32, tag="pp")
                            proj_matmuls(
                                ps,
                                wq_r[:, :, do * P:(do + 1) * P],
                                xq_r[:, :, th * 512:(th + 1) * 512])
                            nc.vector.tensor_copy(
                                qt_r[:, do, th * 512:(th + 1) * 512], ps[:])

                # ---- Phase KV (fused, streaming x^T chunks) ----
                with (
                    tc.tile_pool(name="wkpool", bufs=1) as wkpool,
                    tc.tile_pool(name="wvpool", bufs=1) as wvpool,
                    tc.tile_pool(name="xtrot", bufs=3) as xtrot,
                    tc.tile_pool(name="kost", bufs=4) as kost,
                    tc.tile_pool(name="psum_vv", bufs=4, space="PSUM") as psum_vv,
                    tc.tile_pool(name="psum_kk", bufs=4, space="PSUM") as psum_kk,
                ):
                    wv_r = wvpool.tile([P, DC, D], dt.float32r)
                    wk_r = wkpool.tile([P, DC, D], dt.float32r)
                    wva = wv.rearrange("(dc p) m -> p dc m", p=P)
                    wka = wk.rearrange("(dc p) m -> p dc m", p=P)
                    xt_ra = xt.rearrange("(dc p) t -> p dc t", p=P)
                    for h in range(2):
                        sl = slice(h * 512, (h + 1) * 512)
                        nc.sync.dma_start(wv_r[:, :, sl], wva[:, :, sl])
                    xt_c0 = xtrot.tile([P, DC, CH], dt.float32r, tag="xtc",
                                       name="xtc0")
                    nc.sync.dma_start(xt_c0[:], xt_ra[:, :, 0:CH])
                    for h in range(2):
                        sl = slice(h * 512, (h + 1) * 512)
                        nc.sync.dma_start(wk_r[:, :, sl], wka[:, :, sl])
                    for ch in range(S // CH):
                        if ch == 0:
                            xt_c = xt_c0
                        else:
                            xt_c = xtrot.tile([P, DC, CH], dt.float32r,
                                              tag="xtc", name=f"xtc{ch}")
                            nc.sync.dma_start(
                                xt_c[:], xt_ra[:, :, ch * CH:(ch + 1) * CH])
                        # v rows for these 256 keys
                        for j in range(2):
                            kc = 2 * ch + j
                            for dh in range(2):
                                ps = psum_vv.tile([P, 512], dt.float32,
                                                  tag="pv")
                                proj_matmuls(
                                    ps,
                                    xt_c[:, :, j * P:(j + 1) * P],
                                    wv_r[:, :, dh * 512:(dh + 1) * 512])
                                nc.vector.tensor_copy(
                                    v_res[:, kc, dh * 512:(dh + 1) * 512],
                                    ps[:])
                        # k^T panel half (keys ch*256 .. +256)
                        kq, half = ch // 2, ch % 2
                        for do in range(DC):
                            ps = psum_kk.tile([P, CH], dt.float32, tag="pk")
                            proj_matmuls(
                                ps,
                                wk_r[:, :, do * P:(do + 1) * P],
                                xt_c)
                            st = kost.tile([P, CH], dt.float32r, tag="ko")
                            nc.vector.tensor_copy(st[:], ps[:])
                            nc.sync.dma_start(
                                kt_ds[kq][:, do, half * CH:(half + 1) * CH],
                                st[:])

                # ---- Phase A: blockwise masked softmax(QK^T/32) V ----
                with (
                    tc.tile_pool(name="attn", bufs=1) as attn,
                    tc.tile_pool(name="ktpool", bufs=2) as ktpool,
                    tc.tile_pool(name="ptpool", bufs=1) as ptpool,
                    tc.tile_pool(name="opool", bufs=2) as opool,
                    tc.tile_pool(name="small", bufs=24) as small,
                    tc.tile_pool(name="psum_s", bufs=2, space="PSUM") as psum_s,
                    tc.tile_pool(name="psum_t", bufs=2, space="PSUM") as psum_t,
                    tc.tile_pool(name="psum_c", bufs=4, space="PSUM") as psum_c,
                ):
                    ident = attn.tile([P, P], dt.float32)
                    make_identity(nc, ident)
                    masks = attn.tile([P, NSLOT, 2, KPAN], dt.float32)
                    nc.sync.dma_start(masks[:], mb[:])
                    scores = [
                        attn.tile([P, 2, (s + 1) * KPAN], dt.float32,
                                  tag=f"sc{s}", name=f"scores{s}")
                        for s in range(NSLOT)
                    ]
                    # panel-major scores: k^T panel read once
                    for p in range(NSLOT):
                        ktp = ktpool.tile([P, DC, KPAN], dt.float32r, tag="kt")
                        nc.sync.dma_start(ktp[:], kt_ds[p][:])
                        for s in range(p, NSLOT):
                            for qs in range(2):
                                ps = psum_s.tile([P, KPAN], dt.float32,
                                                 tag="ps")
                                for dc in range(DC):
                                    nc.tensor.matmul(
                                        ps,
                                        qt_r[:, dc,
                                             s * QBLK + qs * P:
                                             s * QBLK + (qs + 1) * P],
                                        ktp[:, dc],
                                        start=(dc == 0), stop=(dc == DC - 1),
                                    )
                                dst = scores[s][:, qs, p * KPAN:(p + 1) * KPAN]
                                if p == s:  # this slot's last panel: add mask
                                    nc.vector.tensor_tensor(
                                        dst, ps[:], masks[:, s, qs, :],
                                        op=mybir.AluOpType.add)
                                else:
                                    nc.vector.tensor_copy(dst, ps[:])

                    for s in range(NSLOT):
                        W = (s + 1) * KPAN
                        KC = W // P
                        rinvs = []
                        for qs in range(2):
                            row = scores[s][:, qs, :]
                            mx = small.tile([P, 1], dt.float32, tag="mx")
                            nc.vector.reduce_max(
                                mx, row, axis=mybir.AxisListType.X)
                            bias_act = small.tile([P, 1], dt.float32, tag="ba")
                            nc.vector.tensor_scalar_mul(bias_act, mx, -SCALE)
                            lsum = small.tile([P, 1], dt.float32, tag="ls")
                            nc.scalar.activation(
                                out=row, in_=row,
                                func=mybir.ActivationFunctionType.Exp,
                                bias=bias_act, scale=SCALE, accum_out=lsum)
                            rinv = small.tile([P, 1], dt.float32, tag="ri")
                            nc.vector.reciprocal(rinv, lsum)
                            rinvs.append(rinv)
                        # transpose p -> pT (f32r) for the AV matmul
                        pt = ptpool.tile([P, 16, QBLK], dt.float32r, tag="pt")
                        for kc in range(KC):
                            for qs in range(2):
                                tps = psum_t.tile([P, P], dt.float32, tag="tp")
                                nc.tensor.transpose(
                                    tps,
                                    scores[s][:, qs, kc * P:(kc + 1) * P],
                                    ident)
                                nc.vector.tensor_copy(
                                    pt[:, kc, qs * P:(qs + 1) * P], tps[:])
                        # AV: ctx[q, dout] accumulated over key chunks
                        ctx = [[psum_c.tile([P, 512], dt.float32, tag="ctx",
                                             name=f"ctx{s}_{qs}_{dh}")
                                for dh in range(2)] for qs in range(2)]
                        for kc in range(KC):
                            for qs in range(2):
                                for dh in range(2):
                                    nc.tensor.matmul(
                                        ctx[qs][dh],
                                        pt[:, kc, qs * P:(qs + 1) * P],
                                        v_res[:, kc, dh * 512:(dh + 1) * 512],
                                        start=(kc == 0), stop=(kc == KC - 1),
                                    )
                        for qs in range(2):
                            for dh in range(2):
                                oc = opool.tile([P, 512], dt.float32, tag="oc")
                                nc.vector.tensor_tensor(
                                    oc[:], ctx[qs][dh],
                                    rinvs[qs][:].to_broadcast((P, 512)),
                                    op=mybir.AluOpType.mult)
                                nc.sync.dma_start(
                                    out[s * QBLK + qs * P:
                                        s * QBLK + (qs + 1) * P,
                                        dh * 512:(dh + 1) * 512],
                                    oc[:])

            if reps > 1:
                with tc.For_i(0, reps):
                    body()
            else:
                body()

    nc.finalize()
    return nc


def make_core_inputs(x, Wq, Wk, Wv):
    """Slice/transform full inputs into 8 per-core input dicts."""
    in_maps = []
    wq_r, wk_r, wv_r = round_f32r(Wq), round_f32r(Wk), round_f32r(Wv)
    qi = np.arange(QBLK)
    for c in range(8):
        b, par = c // 2, c % 2
        blocks = [2 * j + par for j in range(NSLOT)]
        xb = x[b]  # [S, D]
        xt = np.ascontiguousarray(xb.T)  # [D, S]
        qrows = np.concatenate(
            [np.arange(QBLK * blk, QBLK * (blk + 1)) for blk in blocks])
        xqt = np.ascontiguousarray(xb[qrows].T)  # [D, 1024]
        # additive bias for each slot's last key panel
        mb = np.zeros((NSLOT, 2, P, KPAN), np.float32)
        for s in range(NSLOT):
            bs = blocks[s]
            kidx = (COUNTS[s] - 1) * KPAN + np.arange(KPAN)[None, :]
            qidx = (QBLK * bs + qi)[:, None]
            bias = np.where(kidx <= qidx, 0.0, NEG).astype(np.float32)
            mb[s] = bias.reshape(2, P, KPAN)
        mb = np.ascontiguousarray(mb.transpose(2, 0, 1, 3))  # [P, slot, qs, k]
        in_maps.append({
            "xt": round_f32r(xt), "xqt": round_f32r(xqt),
            "wq": wq_r, "wk": wk_r, "wv": wv_r, "mb": mb,
        })
    return in_maps


def assemble_output(results):
    out = np.empty((B, S, D), np.float32)
    for c in range(8):
        b, par = c // 2, c % 2
        blocks = [2 * j + par for j in range(NSLOT)]
        o = results[c]["out"]  # [1024, D]
        for s, blk in enumerate(blocks):
            out[b, QBLK * blk:QBLK * (blk + 1)] = o[QBLK * s:QBLK * (s + 1)]
    return out


def kernel(x, Wq, Wk, Wv):
    x = np.asarray(x, np.float32)
    Wq = np.asarray(Wq, np.float32)
    Wk = np.asarray(Wk, np.float32)
    Wv = np.asarray(Wv, np.float32)
    if "nc" not in _nc_cache:
        _nc_cache["nc"] = build_nc()
    nc = _nc_cache["nc"]
    in_maps = make_core_inputs(x, Wq, Wk, Wv)
    res = run_bass_kernel_spmd(nc, in_maps, core_ids=list(range(8)))
    return assemble_output(res.results)


# revision 16
# speedup vs baseline: 1.2726x; 1.2176x over previous
"""Causal single-head attention on 8 TRN2 NeuronCores.

Problem: x[4, 2048, 1024], Wq/Wk/Wv[1024, 1024] fp32.
  q,k,v = x@W*; scores = q@k^T; masked = scores*tril + (1-tril)*(-1e9)
  attn = softmax(masked/sqrt(1024)); out = attn@v.

Sharding: 2 cores per batch. Query rows are split into eight 256-row
blocks; parity-0 cores take blocks {0,2,4,6}, parity-1 {1,3,5,7}, so
each core's 4 slots attend to exactly (1,2,3,4) 512-wide key panels —
identical program on all 8 cores (SPMD), balanced causal work, no
collectives. Each core computes k/v projections for its whole batch
(k^T and v bounce through DRAM scratch), q projection for its 1024
rows, then block-wise masked softmax(QK^T/32)V. Matmuls run in
float32r (~13-bit mantissa, 4x faster than fp32 on the PE).

Host side: slices x per core, pre-transposes x and xq (so the kernel
needs no PE transposes for projections), builds additive causal mask
biases for each slot's last key panel, and scatters the per-core
outputs back into the full [4, 2048, 1024] tensor.
"""
import sys

if "/opt/trn_rl_repo" not in sys.path:
    sys.path.insert(0, "/opt/trn_rl_repo")

import numpy as np

import concourse.bass as bass
import concourse.tile as tile
from concourse import bacc, mybir
from concourse.bass_utils import run_bass_kernel_spmd
from concourse.masks import make_identity

dt = mybir.dt

B, S, D = 4, 2048, 1024
P = 128
NEG = -1.0e9
QBLK = 256            # query rows per slot
KPAN = 512            # key panel width
NSLOT = 4             # slots per core
COUNTS = (1, 2, 3, 4)  # key panels per slot (both parities)
SCALE = 1.0 / 32.0    # 1/sqrt(D)

_nc_cache = {}


def round_f32r(a):
    """Host replica of the DVE fp32->float32r rounding: round-to-nearest-even
    to 11 mantissa bits (drop 12). Verified bit-exact vs hardware."""
    u = np.ascontiguousarray(a, np.float32).view(np.uint32).astype(np.uint64)
    half = np.uint64(1 << 11)
    tie = ((u >> np.uint64(12)) & np.uint64(1)) ^ np.uint64(1)
    r = (u + half - tie) & np.uint64(0xFFFFF000)
    return r.astype(np.uint32).view(np.float32)


def build_nc(reps=1):
    """Build the per-core Bass program (same NEFF for all 8 cores).

    All matmuls run in float32r. The host pre-rounds every input to f32r
    bits, so inputs DMA straight into f32r tiles with no on-device
    rounding pass. Phases: Q (q^T, kept resident) -> fused K+V streaming
    over x^T chunks (k^T panels bounce through DRAM, v resident) ->
    panel-major masked softmax(QK^T/32) V.
    """
    nc = bacc.Bacc(None, target_bir_lowering=False, debug=False)

    # all big inputs arrive pre-rounded to f32r bit patterns
    xt = nc.dram_tensor("xt", [D, S], dt.float32r, kind="ExternalInput")
    xqt = nc.dram_tensor("xqt", [D, NSLOT * QBLK], dt.float32r,
                         kind="ExternalInput")
    wq = nc.dram_tensor("wq", [D, D], dt.float32r, kind="ExternalInput")
    wk = nc.dram_tensor("wk", [D, D], dt.float32r, kind="ExternalInput")
    wv = nc.dram_tensor("wv", [D, D], dt.float32r, kind="ExternalInput")
    # additive causal bias for each slot's LAST key panel, laid out
    # [p, slot, qsub, key] with q-local row = qsub*128 + p
    mb = nc.dram_tensor("mb", [P, NSLOT, 2, KPAN], dt.float32,
                        kind="ExternalInput")
    out = nc.dram_tensor("out", [NSLOT * QBLK, D], dt.float32,
                         kind="ExternalOutput")

    # k^T DRAM bounce, one tensor per 512-key panel (fine-grained deps)
    kt_ds = [nc.dram_tensor(f"kt_d{p}", [P, 8, KPAN], dt.float32r)
             for p in range(NSLOT)]

    DC = D // P  # 8 contraction chunks
    CH = 256     # x^T streaming chunk width (keys)

    def proj_matmuls(psum_t, lhs_r, rhs_r):
        for dc in range(DC):
            nc.tensor.matmul(
                psum_t, lhs_r[:, dc], rhs_r[:, dc],
                start=(dc == 0), stop=(dc == DC - 1),
            )

    with tile.TileContext(nc) as tc:
        with (
            tc.tile_pool(name="vres", bufs=1) as vres,
            tc.tile_pool(name="qtres", bufs=1) as qtres,
        ):
            # v[key, dout] and q^T, resident through the attention phase
            v_res = vres.tile([P, S // P, D], dt.float32r)
            qt_r = qtres.tile([P, DC, NSLOT * QBLK], dt.float32r)

            def body():
                from contextlib import ExitStack
                tcx = ExitStack()
                # ---- Phase Q: q^T -> qt_r (SBUF resident) ----
                with (
                    tc.tile_pool(name="wqpool", bufs=1) as wqpool,
                    tc.tile_pool(name="xqpool", bufs=1) as xqpool,
                    tc.tile_pool(name="psum_q", bufs=4, space="PSUM") as psum_q,
                ):
                    wq_r = wqpool.tile([P, DC, D], dt.float32r)
                    xq_r = xqpool.tile([P, DC, NSLOT * QBLK], dt.float32r)
                    wqa = wq.rearrange("(dc p) m -> p dc m", p=P)
                    xqa = xqt.rearrange("(dc p) t -> p dc t", p=P)
                    nc.sync.dma_start(xq_r[:, :, 0:512], xqa[:, :, 0:512])
                    for do in range(DC):
                        sl = slice(do * P, (do + 1) * P)
                        nc.sync.dma_start(wq_r[:, :, sl], wqa[:, :, sl])
                    nc.sync.dma_start(xq_r[:, :, 512:1024], xqa[:, :, 512:1024])
                    for th in range(2):
                        for do in range(DC):
                            ps = psum_q.tile([P, 512], dt.float32, tag="pp")
                            proj_matmuls(
                                ps,
                                wq_r[:, :, do * P:(do + 1) * P],
                                xq_r[:, :, th * 512:(th + 1) * 512])
                            nc.vector.tensor_copy(
                                qt_r[:, do, th * 512:(th + 1) * 512], ps[:])

                # ---- Phase KV (fused, streaming x^T chunks) ----
                # ktpool/psum_s opened first: reserved below the KV pools so
                # the attention phase's first k^T panel load and score psums
                # carry no WAR dependency on KV-phase memory
                ktpool = tcx.enter_context(tc.tile_pool(name="ktpool", bufs=1))
                psum_s = tcx.enter_context(
                    tc.tile_pool(name="psum_s", bufs=2, space="PSUM"))
                with (
                    tc.tile_pool(name="wkpool", bufs=1) as wkpool,
                    tc.tile_pool(name="wvpool", bufs=1) as wvpool,
                    tc.tile_pool(name="xtrot", bufs=3) as xtrot,
                    tc.tile_pool(name="kost", bufs=4) as kost,
                    tc.tile_pool(name="psum_vv", bufs=3, space="PSUM") as psum_vv,
                    tc.tile_pool(name="psum_kk", bufs=3, space="PSUM") as psum_kk,
                ):
                    wv_r = wvpool.tile([P, DC, D], dt.float32r)
                    wk_r = wkpool.tile([P, DC, D], dt.float32r)
                    wva = wv.rearrange("(dc p) m -> p dc m", p=P)
                    wka = wk.rearrange("(dc p) m -> p dc m", p=P)
                    xt_ra = xt.rearrange("(dc p) t -> p dc t", p=P)
                    for h in range(2):
                        sl = slice(h * 512, (h + 1) * 512)
                        nc.sync.dma_start(wv_r[:, :, sl], wva[:, :, sl])
                    xt_c0 = xtrot.tile([P, DC, CH], dt.float32r, tag="xtc",
                                       name="xtc0")
                    nc.sync.dma_start(xt_c0[:], xt_ra[:, :, 0:CH])
                    for h in range(2):
                        sl = slice(h * 512, (h + 1) * 512)
                        nc.sync.dma_start(wk_r[:, :, sl], wka[:, :, sl])
                    for ch in range(S // CH):
                        if ch == 0:
                            xt_c = xt_c0
                        else:
                            xt_c = xtrot.tile([P, DC, CH], dt.float32r,
                                              tag="xtc", name=f"xtc{ch}")
                            nc.sync.dma_start(
                                xt_c[:], xt_ra[:, :, ch * CH:(ch + 1) * CH])
                        # v rows for these 256 keys
                        for j in range(2):
                            kc = 2 * ch + j
                            for dh in range(2):
                                ps = psum_vv.tile([P, 512], dt.float32,
                                                  tag="pv")
                                proj_matmuls(
                                    ps,
                                    xt_c[:, :, j * P:(j + 1) * P],
                                    wv_r[:, :, dh * 512:(dh + 1) * 512])
                                nc.vector.tensor_copy(
                                    v_res[:, kc, dh * 512:(dh + 1) * 512],
                                    ps[:])
                        # k^T panel half (keys ch*256 .. +256)
                        kq, half = ch // 2, ch % 2
                        for do in range(DC):
                            ps = psum_kk.tile([P, CH], dt.float32, tag="pk")
                            proj_matmuls(
                                ps,
                                wk_r[:, :, do * P:(do + 1) * P],
                                xt_c)
                            st = kost.tile([P, CH], dt.float32r, tag="ko")
                            nc.vector.tensor_copy(st[:], ps[:])
                            nc.sync.dma_start(
                                kt_ds[kq][:, do, half * CH:(half + 1) * CH],
                                st[:])

                # ---- Phase A: blockwise masked softmax(QK^T/32) V ----
                with (
                    tc.tile_pool(name="attn", bufs=1) as attn,
                    tc.tile_pool(name="ptpool", bufs=1) as ptpool,
                    tc.tile_pool(name="opool", bufs=2) as opool,
                    tc.tile_pool(name="small", bufs=24) as small,
                    tc.tile_pool(name="psum_t", bufs=2, space="PSUM") as psum_t,
                    tc.tile_pool(name="psum_c", bufs=4, space="PSUM") as psum_c,
                ):
                    ident = attn.tile([P, P], dt.float32)
                    make_identity(nc, ident)
                    masks = attn.tile([P, NSLOT, 2, KPAN], dt.float32)
                    for s in range(NSLOT):
                        nc.gpsimd.dma_start(masks[:, s], mb[:, s])
                    scores = [
                        attn.tile([P, 2, (s + 1) * KPAN], dt.float32,
                                  tag=f"sc{s}", name=f"scores{s}")
                        for s in range(NSLOT)
                    ]
                    # panel-major scores: k^T panel read once
                    for p in range(NSLOT):
                        ktp = ktpool.tile([P, DC, KPAN], dt.float32r, tag="kt")
                        nc.sync.dma_start(ktp[:], kt_ds[p][:])
                        for s in range(p, NSLOT):
                            for qs in range(2):
                                ps = psum_s.tile([P, KPAN], dt.float32,
                                                 tag="ps")
                                for dc in range(DC):
                                    nc.tensor.matmul(
                                        ps,
                                        qt_r[:, dc,
                                             s * QBLK + qs * P:
                                             s * QBLK + (qs + 1) * P],
                                        ktp[:, dc],
                                        start=(dc == 0), stop=(dc == DC - 1),
                                    )
                                dst = scores[s][:, qs, p * KPAN:(p + 1) * KPAN]
                                if p == s:  # this slot's last panel: add mask
                                    nc.vector.tensor_tensor(
                                        dst, ps[:], masks[:, s, qs, :],
                                        op=mybir.AluOpType.add)
                                else:
                                    nc.vector.tensor_copy(dst, ps[:])

                    for s in range(NSLOT):
                        W = (s + 1) * KPAN
                        KC = W // P
                        rinvs = []
                        for qs in range(2):
                            row = scores[s][:, qs, :]
                            mx = small.tile([P, 1], dt.float32, tag="mx")
                            nc.vector.reduce_max(
                                mx, row, axis=mybir.AxisListType.X)
                            bias_act = small.tile([P, 1], dt.float32, tag="ba")
                            nc.vector.tensor_scalar_mul(bias_act, mx, -SCALE)
                            lsum = small.tile([P, 1], dt.float32, tag="ls")
                            nc.scalar.activation(
                                out=row, in_=row,
                                func=mybir.ActivationFunctionType.Exp,
                                bias=bias_act, scale=SCALE, accum_out=lsum)
                            rinv = small.tile([P, 1], dt.float32, tag="ri")
                            nc.vector.reciprocal(rinv, lsum)
                            rinvs.append(rinv)
                        # transpose p -> pT (f32r) for the AV matmul
                        pt = ptpool.tile([P, 16, QBLK], dt.float32r, tag="pt")
                        for kc in range(KC):
                            tps = psum_t.tile([P, 2, P], dt.float32, tag="tp")
                            for qs in range(2):
                                nc.tensor.transpose(
                                    tps[:, qs],
                                    scores[s][:, qs, kc * P:(kc + 1) * P],
                                    ident)
                            nc.vector.tensor_copy(pt[:, kc, :], tps[:])
                        # AV: ctx[q, dout]; kc-inner chains so each
                        # (qs, dh) output drains as soon as its chain ends
                        for qs in range(2):
                            for dh in range(2):
                                ctx = psum_c.tile([P, 512], dt.float32,
                                                  tag="ctx",
                                                  name=f"ctx{s}_{qs}_{dh}")
                                for kc in range(KC):
                                    nc.tensor.matmul(
                                        ctx,
                                        pt[:, kc, qs * P:(qs + 1) * P],
                                        v_res[:, kc, dh * 512:(dh + 1) * 512],
                                        start=(kc == 0), stop=(kc == KC - 1),
                                    )
                                oc = opool.tile([P, 512], dt.float32, tag="oc")
                                nc.vector.tensor_tensor(
                                    oc[:], ctx,
                                    rinvs[qs][:].to_broadcast((P, 512)),
                                    op=mybir.AluOpType.mult)
                                nc.sync.dma_start(
                                    out[s * QBLK + qs * P:
                                        s * QBLK + (qs + 1) * P,
                                        dh * 512:(dh + 1) * 512],
                                    oc[:])
                tcx.close()

            if reps > 1:
                for _ in range(reps):
                    body()
            else:
                body()

    nc.finalize()
    return nc


def make_core_inputs(x, Wq, Wk, Wv):
    """Slice/transform full inputs into 8 per-core input dicts."""
    in_maps = []
    wq_r, wk_r, wv_r = round_f32r(Wq), round_f32r(Wk), round_f32r(Wv)
    qi = np.arange(QBLK)
    for c in range(8):
        b, par = c // 2, c % 2
        blocks = [2 * j + par for j in range(NSLOT)]
        xb = x[b]  # [S, D]
        xt = np.ascontiguousarray(xb.T)  # [D, S]
        qrows = np.concatenate(
            [np.arange(QBLK * blk, QBLK * (blk + 1)) for blk in blocks])
        xqt = np.ascontiguousarray(xb[qrows].T)  # [D, 1024]
        # additive bias for each slot's last key panel
        mb = np.zeros((NSLOT, 2, P, KPAN), np.float32)
        for s in range(NSLOT):
            bs = blocks[s]
            kidx = (COUNTS[s] - 1) * KPAN + np.arange(KPAN)[None, :]
            qidx = (QBLK * bs + qi)[:, None]
            bias = np.where(kidx <= qidx, 0.0, NEG).astype(np.float32)
            mb[s] = bias.reshape(2, P, KPAN)
        mb = np.ascontiguousarray(mb.transpose(2, 0, 1, 3))  # [P, slot, qs, k]
        in_maps.append({
            "xt": round_f32r(xt), "xqt": round_f32r(xqt),
            "wq": wq_r, "wk": wk_r, "wv": wv_r, "mb": mb,
        })
    return in_maps


def assemble_output(results):
    out = np.empty((B, S, D), np.float32)
    for c in range(8):
        b, par = c // 2, c % 2
        blocks = [2 * j + par for j in range(NSLOT)]
        o = results[c]["out"]  # [1024, D]
        for s, blk in enumerate(blocks):
            out[b, QBLK * blk:QBLK * (blk + 1)] = o[QBLK * s:QBLK * (s + 1)]
    return out


def kernel(x, Wq, Wk, Wv):
    x = np.asarray(x, np.float32)
    Wq = np.asarray(Wq, np.float32)
    Wk = np.asarray(Wk, np.float32)
    Wv = np.asarray(Wv, np.float32)
    if "nc" not in _nc_cache:
        _nc_cache["nc"] = build_nc()
    nc = _nc_cache["nc"]
    in_maps = make_core_inputs(x, Wq, Wk, Wv)
    res = run_bass_kernel_spmd(nc, in_maps, core_ids=list(range(8)))
    return assemble_output(res.results)


# revision 17
# speedup vs baseline: 1.4643x; 1.1506x over previous
"""Causal single-head attention on 8 TRN2 NeuronCores.

Problem: x[4, 2048, 1024], Wq/Wk/Wv[1024, 1024] fp32.
  q,k,v = x@W*; scores = q@k^T; masked = scores*tril + (1-tril)*(-1e9)
  attn = softmax(masked/sqrt(1024)); out = attn@v.

Sharding: 2 cores per batch. Query rows are split into eight 256-row
blocks; parity-0 cores take blocks {0,2,4,6}, parity-1 {1,3,5,7}, so
each core's 4 slots attend to exactly (1,2,3,4) 512-wide key panels —
identical program on all 8 cores (SPMD), balanced causal work, no
collectives. Each core computes k/v projections for its whole batch
(k^T and v bounce through DRAM scratch), q projection for its 1024
rows, then block-wise masked softmax(QK^T/32)V. Matmuls run in
float32r (~13-bit mantissa, 4x faster than fp32 on the PE).

Host side: slices x per core, pre-transposes x and xq (so the kernel
needs no PE transposes for projections), builds additive causal mask
biases for each slot's last key panel, and scatters the per-core
outputs back into the full [4, 2048, 1024] tensor.
"""
import sys

if "/opt/trn_rl_repo" not in sys.path:
    sys.path.insert(0, "/opt/trn_rl_repo")

import numpy as np

import concourse.bass as bass
import concourse.tile as tile
from concourse import bacc, mybir
from concourse.bass_utils import run_bass_kernel_spmd
from concourse.masks import make_identity

dt = mybir.dt

B, S, D = 4, 2048, 1024
P = 128
NEG = -1.0e9
QBLK = 256            # query rows per slot
KPAN = 512            # key panel width
NSLOT = 4             # slots per core
COUNTS = (1, 2, 3, 4)  # key panels per slot (both parities)
SCALE = 1.0 / 32.0    # 1/sqrt(D)

_nc_cache = {}


def round_f32r(a):
    """Host replica of the DVE fp32->float32r rounding: round-to-nearest-even
    to 11 mantissa bits (drop 12). Verified bit-exact vs hardware."""
    u = np.ascontiguousarray(a, np.float32).view(np.uint32).astype(np.uint64)
    half = np.uint64(1 << 11)
    tie = ((u >> np.uint64(12)) & np.uint64(1)) ^ np.uint64(1)
    r = (u + half - tie) & np.uint64(0xFFFFF000)
    return r.astype(np.uint32).view(np.float32)


def build_nc(reps=1):
    """Build the per-core Bass program (same NEFF for all 8 cores).

    All matmuls run in float32r. The host pre-rounds every input to f32r
    bits, so inputs DMA straight into f32r tiles with no on-device
    rounding pass. Phases: Q (q^T, kept resident) -> fused K+V streaming
    over x^T chunks (k^T panels bounce through DRAM, v resident) ->
    panel-major masked softmax(QK^T/32) V.
    """
    nc = bacc.Bacc(None, target_bir_lowering=False, debug=False)

    # all big inputs arrive pre-rounded to f32r bit patterns
    xt = nc.dram_tensor("xt", [D, S], dt.float32r, kind="ExternalInput")
    xqt = nc.dram_tensor("xqt", [D, NSLOT * QBLK], dt.float32r,
                         kind="ExternalInput")
    wq = nc.dram_tensor("wq", [D, D], dt.float32r, kind="ExternalInput")
    wk = nc.dram_tensor("wk", [D, D], dt.float32r, kind="ExternalInput")
    wv = nc.dram_tensor("wv", [D, D], dt.float32r, kind="ExternalInput")
    # additive causal bias for each slot's LAST key panel, laid out
    # [p, slot, qsub, key] with q-local row = qsub*128 + p
    mb = nc.dram_tensor("mb", [P, NSLOT, 2, KPAN], dt.float32,
                        kind="ExternalInput")
    out = nc.dram_tensor("out", [NSLOT * QBLK, D], dt.float32,
                         kind="ExternalOutput")

    # k^T DRAM bounce, one tensor per 512-key panel (fine-grained deps)
    kt_ds = [nc.dram_tensor(f"kt_d{p}", [P, 8, KPAN], dt.float32r)
             for p in range(NSLOT)]

    DC = D // P  # 8 contraction chunks
    CH = 256     # x^T streaming chunk width (keys)

    def proj_matmuls(psum_t, lhs_r, rhs_r):
        for dc in range(DC):
            nc.tensor.matmul(
                psum_t, lhs_r[:, dc], rhs_r[:, dc],
                start=(dc == 0), stop=(dc == DC - 1),
            )

    with tile.TileContext(nc) as tc:
        with (
            tc.tile_pool(name="vres", bufs=1) as vres,
            tc.tile_pool(name="qtres", bufs=1) as qtres,
        ):
            # v[key, dout] and q^T, resident through the attention phase
            v_res = vres.tile([P, S // P, D], dt.float32r)
            qt_r = qtres.tile([P, DC, NSLOT * QBLK], dt.float32r)

            def body():
                from contextlib import ExitStack
                tcx = ExitStack()
                # ---- Phase Q: q^T -> qt_r (SBUF resident) ----
                with (
                    tc.tile_pool(name="wqpool", bufs=1) as wqpool,
                    tc.tile_pool(name="xqpool", bufs=1) as xqpool,
                    tc.tile_pool(name="psum_q", bufs=4, space="PSUM") as psum_q,
                ):
                    wq_r = wqpool.tile([P, DC, D], dt.float32r)
                    xq_r = xqpool.tile([P, DC, NSLOT * QBLK], dt.float32r)
                    wqa = wq.rearrange("(dc p) m -> p dc m", p=P)
                    xqa = xqt.rearrange("(dc p) t -> p dc t", p=P)
                    nc.sync.dma_start(xq_r[:, :, 0:512], xqa[:, :, 0:512])
                    for do in range(DC):
                        sl = slice(do * P, (do + 1) * P)
                        nc.sync.dma_start(wq_r[:, :, sl], wqa[:, :, sl])
                    nc.sync.dma_start(xq_r[:, :, 512:1024], xqa[:, :, 512:1024])
                    for th in range(2):
                        for do in range(DC):
                            ps = psum_q.tile([P, 512], dt.float32, tag="pp")
                            proj_matmuls(
                                ps,
                                wq_r[:, :, do * P:(do + 1) * P],
                                xq_r[:, :, th * 512:(th + 1) * 512])
                            nc.vector.tensor_copy(
                                qt_r[:, do, th * 512:(th + 1) * 512], ps[:])

                # ---- Phase KV (fused, streaming x^T chunks) ----
                # ktpool/psum_s opened first: reserved below the KV pools so
                # the attention phase's first k^T panel load and score psums
                # carry no WAR dependency on KV-phase memory
                ktpool = tcx.enter_context(tc.tile_pool(name="ktpool", bufs=1))
                psum_s = tcx.enter_context(
                    tc.tile_pool(name="psum_s", bufs=2, space="PSUM"))
                with (
                    tc.tile_pool(name="wkpool", bufs=1) as wkpool,
                    tc.tile_pool(name="wvpool", bufs=1) as wvpool,
                    tc.tile_pool(name="xtrot", bufs=3) as xtrot,
                    tc.tile_pool(name="kost", bufs=4) as kost,
                    tc.tile_pool(name="psum_vv", bufs=3, space="PSUM") as psum_vv,
                    tc.tile_pool(name="psum_kk", bufs=3, space="PSUM") as psum_kk,
                ):
                    wv_r = wvpool.tile([P, DC, D], dt.float32r)
                    wk_r = wkpool.tile([P, DC, D], dt.float32r)
                    wva = wv.rearrange("(dc p) m -> p dc m", p=P)
                    wka = wk.rearrange("(dc p) m -> p dc m", p=P)
                    xt_ra = xt.rearrange("(dc p) t -> p dc t", p=P)
                    # order: first V matmul needs only wv[:, :, :512] + chunk 0
                    nc.sync.dma_start(wv_r[:, :, 0:512], wva[:, :, 0:512])
                    xt_c0 = xtrot.tile([P, DC, CH], dt.float32r, tag="xtc",
                                       name="xtc0")
                    nc.sync.dma_start(xt_c0[:], xt_ra[:, :, 0:CH])
                    nc.sync.dma_start(wv_r[:, :, 512:1024], wva[:, :, 512:1024])
                    for h in range(2):
                        sl = slice(h * 512, (h + 1) * 512)
                        nc.sync.dma_start(wk_r[:, :, sl], wka[:, :, sl])
                    for ch in range(S // CH):
                        if ch == 0:
                            xt_c = xt_c0
                        else:
                            xt_c = xtrot.tile([P, DC, CH], dt.float32r,
                                              tag="xtc", name=f"xtc{ch}")
                            nc.sync.dma_start(
                                xt_c[:], xt_ra[:, :, ch * CH:(ch + 1) * CH])
                        # v rows for these 256 keys
                        for j in range(2):
                            kc = 2 * ch + j
                            for dh in range(2):
                                ps = psum_vv.tile([P, 512], dt.float32,
                                                  tag="pv")
                                proj_matmuls(
                                    ps,
                                    xt_c[:, :, j * P:(j + 1) * P],
                                    wv_r[:, :, dh * 512:(dh + 1) * 512])
                                nc.vector.tensor_copy(
                                    v_res[:, kc, dh * 512:(dh + 1) * 512],
                                    ps[:])
                        # k^T panel half (keys ch*256 .. +256)
                        kq, half = ch // 2, ch % 2
                        for do in range(DC):
                            ps = psum_kk.tile([P, CH], dt.float32, tag="pk")
                            proj_matmuls(
                                ps,
                                wk_r[:, :, do * P:(do + 1) * P],
                                xt_c)
                            st = kost.tile([P, CH], dt.float32r, tag="ko")
                            nc.vector.tensor_copy(st[:], ps[:])
                            nc.sync.dma_start(
                                kt_ds[kq][:, do, half * CH:(half + 1) * CH],
                                st[:])

                # ---- Phase A: blockwise masked softmax(QK^T/32) V ----
                with (
                    tc.tile_pool(name="attn", bufs=1) as attn,
                    tc.tile_pool(name="ptpool", bufs=1) as ptpool,
                    tc.tile_pool(name="opool", bufs=2) as opool,
                    tc.tile_pool(name="small", bufs=24) as small,
                    tc.tile_pool(name="psum_t", bufs=2, space="PSUM") as psum_t,
                    tc.tile_pool(name="psum_c", bufs=4, space="PSUM") as psum_c,
                ):
                    ident = attn.tile([P, P], dt.float32)
                    make_identity(nc, ident)
                    masks = attn.tile([P, NSLOT, 2, KPAN], dt.float32)
                    for s in range(NSLOT):
                        nc.gpsimd.dma_start(masks[:, s], mb[:, s])
                    scores = [
                        attn.tile([P, 2, (s + 1) * KPAN], dt.float32,
                                  tag=f"sc{s}", name=f"scores{s}")
                        for s in range(NSLOT)
                    ]
                    # panel-major scores: k^T panel read once
                    for p in range(NSLOT):
                        ktp = ktpool.tile([P, DC, KPAN], dt.float32r, tag="kt")
                        nc.sync.dma_start(ktp[:], kt_ds[p][:])
                        for s in range(p, NSLOT):
                            for qs in range(2):
                                ps = psum_s.tile([P, KPAN], dt.float32,
                                                 tag="ps")
                                for dc in range(DC):
                                    nc.tensor.matmul(
                                        ps,
                                        qt_r[:, dc,
                                             s * QBLK + qs * P:
                                             s * QBLK + (qs + 1) * P],
                                        ktp[:, dc],
                                        start=(dc == 0), stop=(dc == DC - 1),
                                    )
                                dst = scores[s][:, qs, p * KPAN:(p + 1) * KPAN]
                                if p == s:  # this slot's last panel: add mask
                                    nc.vector.tensor_tensor(
                                        dst, ps[:], masks[:, s, qs, :],
                                        op=mybir.AluOpType.add)
                                else:
                                    nc.vector.tensor_copy(dst, ps[:])

                    for s in range(NSLOT):
                        W = (s + 1) * KPAN
                        KC = W // P
                        rinvs = []
                        for qs in range(2):
                            row = scores[s][:, qs, :]
                            mx = small.tile([P, 1], dt.float32, tag="mx")
                            nc.vector.reduce_max(
                                mx, row, axis=mybir.AxisListType.X)
                            bias_act = small.tile([P, 1], dt.float32, tag="ba")
                            nc.vector.tensor_scalar_mul(bias_act, mx, -SCALE)
                            lsum = small.tile([P, 1], dt.float32, tag="ls")
                            nc.scalar.activation(
                                out=row, in_=row,
                                func=mybir.ActivationFunctionType.Exp,
                                bias=bias_act, scale=SCALE, accum_out=lsum)
                            rinv = small.tile([P, 1], dt.float32, tag="ri")
                            nc.vector.reciprocal(rinv, lsum)
                            rinvs.append(rinv)
                        # transpose p -> pT (f32r) for the AV matmul
                        pt = ptpool.tile([P, 16, QBLK], dt.float32r, tag="pt")
                        for kc in range(KC):
                            tps = psum_t.tile([P, 2, P], dt.float32, tag="tp")
                            for qs in range(2):
                                nc.tensor.transpose(
                                    tps[:, qs],
                                    scores[s][:, qs, kc * P:(kc + 1) * P],
                                    ident)
                            nc.vector.tensor_copy(pt[:, kc, :], tps[:])
                        # AV: ctx[q, dout]; kc-inner chains so each
                        # (qs, dh) output drains as soon as its chain ends
                        for qs in range(2):
                            for dh in range(2):
                                ctx = psum_c.tile([P, 512], dt.float32,
                                                  tag="ctx",
                                                  name=f"ctx{s}_{qs}_{dh}")
                                for kc in range(KC):
                                    nc.tensor.matmul(
                                        ctx,
                                        pt[:, kc, qs * P:(qs + 1) * P],
                                        v_res[:, kc, dh * 512:(dh + 1) * 512],
                                        start=(kc == 0), stop=(kc == KC - 1),
                                    )
                                oc = opool.tile([P, 512], dt.float32, tag="oc")
                                nc.vector.tensor_tensor(
                                    oc[:], ctx,
                                    rinvs[qs][:].to_broadcast((P, 512)),
                                    op=mybir.AluOpType.mult)
                                nc.sync.dma_start(
                                    out[s * QBLK + qs * P:
                                        s * QBLK + (qs + 1) * P,
                                        dh * 512:(dh + 1) * 512],
                                    oc[:])
                tcx.close()

            if reps > 1:
                for _ in range(reps):
                    body()
            else:
                body()

    nc.finalize()
    return nc


def make_core_inputs(x, Wq, Wk, Wv):
    """Slice/transform full inputs into 8 per-core input dicts."""
    in_maps = []
    wq_r, wk_r, wv_r = round_f32r(Wq), round_f32r(Wk), round_f32r(Wv)
    qi = np.arange(QBLK)
    for c in range(8):
        b, par = c // 2, c % 2
        blocks = [2 * j + par for j in range(NSLOT)]
        xb = x[b]  # [S, D]
        xt = np.ascontiguousarray(xb.T)  # [D, S]
        qrows = np.concatenate(
            [np.arange(QBLK * blk, QBLK * (blk + 1)) for blk in blocks])
        xqt = np.ascontiguousarray(xb[qrows].T)  # [D, 1024]
        # additive bias for each slot's last key panel
        mb = np.zeros((NSLOT, 2, P, KPAN), np.float32)
        for s in range(NSLOT):
            bs = blocks[s]
            kidx = (COUNTS[s] - 1) * KPAN + np.arange(KPAN)[None, :]
            qidx = (QBLK * bs + qi)[:, None]
            bias = np.where(kidx <= qidx, 0.0, NEG).astype(np.float32)
            mb[s] = bias.reshape(2, P, KPAN)
        mb = np.ascontiguousarray(mb.transpose(2, 0, 1, 3))  # [P, slot, qs, k]
        in_maps.append({
            "xt": round_f32r(xt), "xqt": round_f32r(xqt),
            "wq": wq_r, "wk": wk_r, "wv": wv_r, "mb": mb,
        })
    return in_maps


def assemble_output(results):
    out = np.empty((B, S, D), np.float32)
    for c in range(8):
        b, par = c // 2, c % 2
        blocks = [2 * j + par for j in range(NSLOT)]
        o = results[c]["out"]  # [1024, D]
        for s, blk in enumerate(blocks):
            out[b, QBLK * blk:QBLK * (blk + 1)] = o[QBLK * s:QBLK * (s + 1)]
    return out


def kernel(x, Wq, Wk, Wv):
    x = np.asarray(x, np.float32)
    Wq = np.asarray(Wq, np.float32)
    Wk = np.asarray(Wk, np.float32)
    Wv = np.asarray(Wv, np.float32)
    if "nc" not in _nc_cache:
        _nc_cache["nc"] = build_nc()
    nc = _nc_cache["nc"]
    in_maps = make_core_inputs(x, Wq, Wk, Wv)
    res = run_bass_kernel_spmd(nc, in_maps, core_ids=list(range(8)))
    return assemble_output(res.results)
